# revision 1
# baseline (speedup 1.0000x reference)
"""TRN2 Bass kernel for nn_LoRA_80839874445852.

Computes out = x @ W^T + b + (x @ A_b) @ B_b / 16 for bs=8, sq=2048, d=4096,
r=16 (per-batch LoRA skill blocks), distributed data-parallel over the batch
dim across 8 NeuronCores.

Strategy:
  * (x @ A) @ B == x @ (A @ B): the rank-16 LoRA term is folded into the
    weight on the host (cheap: 4 distinct [4096,16]@[16,4096] products), so
    each core runs a single dense GEMM:  outT = W_eff^T-contraction with xT.
  * Per core i:  W_eff_i = W.T + A_flat[i//2] @ B_flat[i//2] / 16  [d, o]
  * Device computes outT[o, s] = sum_d W_eff[d, o] * xT[d, s] tiled as
    128x128 stationary (W_eff) x 128x512 moving (xT) bf16 matmuls with fp32
    PSUM accumulation over d (32 tiles), + per-partition bias add on evict.
  * x is resident in SBUF (bf16, 16MB); W_eff streamed once (32x1MB blocks).
  * Host transposes outT back and stacks the full [8, 2048, 4096] output.

bf16 inputs + fp32 accumulate give ~1.6e-3 relative error vs the fp32
reference (error dominated by the input cast; engine accumulation is ~1e-7).
"""
import numpy as np
import ml_dtypes

import concourse.bacc as bacc
import concourse.tile as tile
import concourse.mybir as mybir
from concourse.bass_utils import run_bass_kernel_spmd

# Problem shape (hardcoded per spec)
BS, SQ, D = 8, 2048, 4096
R = 16
N_CORES = 8
P = 128
ND = D // P      # 32 d-tiles (contraction)
NO = D // P      # 32 o-blocks (output features)
NS = 512         # moving free dim per matmul (PSUM bank = 512 fp32)
NST = SQ // NS   # 4 s-tiles

BF16 = mybir.dt.bfloat16
F32 = mybir.dt.float32

_CACHED = {}


def _build():
    """Build + compile the per-core Bass program (same program, all cores)."""
    nc = bacc.Bacc("TRN2", target_bir_lowering=False, debug=False)
    x_d = nc.dram_tensor("x", [ND, P, SQ], BF16, kind="ExternalInput").ap()
    w_d = nc.dram_tensor("w", [NO, P, ND, P], BF16, kind="ExternalInput").ap()
    b_d = nc.dram_tensor("b", [P, NO], F32, kind="ExternalInput").ap()
    y_d = nc.dram_tensor("y", [NO, P, SQ], F32, kind="ExternalOutput").ap()

    with tile.TileContext(nc) as tc:
        with (
            tc.tile_pool(name="xpool", bufs=1) as xpool,
            tc.tile_pool(name="wpool", bufs=3) as wpool,
            tc.tile_pool(name="opool", bufs=3) as opool,
            tc.tile_pool(name="cpool", bufs=1) as cpool,
            tc.tile_pool(name="psum", bufs=2, space="PSUM") as psum_pool,
        ):
            bias_t = cpool.tile([P, NO], F32)
            nc.sync.dma_start(out=bias_t[:], in_=b_d[:])

            # resident x: 32 tiles [128, 2048] bf16 (512KB each)
            xts = []
            for dtile in range(ND):
                xt = xpool.tile([P, SQ], BF16, tag=f"x{dtile}")
                nc.sync.dma_start(out=xt[:], in_=x_d[dtile])
                xts.append(xt)

            for o in range(NO):
                wt = wpool.tile([P, ND, P], BF16, tag="w")
                nc.sync.dma_start(out=wt[:], in_=w_d[o])
                ps = psum_pool.tile([P, NST, NS], F32, tag="ps")
                for d in range(ND):
                    for s_t in range(NST):
                        nc.tensor.matmul(
                            ps[:, s_t],
                            lhsT=wt[:, d],
                            rhs=xts[d][:, s_t * NS:(s_t + 1) * NS],
                            start=(d == 0),
                            stop=(d == ND - 1),
                        )
                ot = opool.tile([P, NST, NS], F32, tag="o")
                nc.vector.tensor_scalar_add(ot[:], ps[:], bias_t[:, o:o + 1])
                nc.sync.dma_start(
                    out=y_d[o], in_=ot[:].rearrange("p nst ns -> p (nst ns)")
                )
    nc.compile()
    return nc


def _prep_inputs(x, W, b, A, B):
    """Host-side shard + layout prep. Returns per-core input maps."""
    x = np.asarray(x, dtype=np.float32)
    W = np.asarray(W, dtype=np.float32)
    b = np.asarray(b, dtype=np.float32)
    A = np.asarray(A, dtype=np.float32)
    B = np.asarray(B, dtype=np.float32)

    n_splits = A.shape[0]
    repeat = BS // n_splits  # 2

    # bias: [128, 32] with bh[p, o] = b[o*128 + p]
    bh = np.ascontiguousarray(b.reshape(NO, P).T)

    # distinct folded weights per skill group, in SBUF tile layout
    w_maps = []
    for g in range(n_splits):
        A_flat = A[g].reshape(D, R)                     # [d, r]
        B_flat = B[g].transpose(1, 0, 2).reshape(R, D)  # [r, o]
        W_eff = W.T + (A_flat @ B_flat) * (1.0 / R)     # [d, o]
        # Wh[o_t, p, do, q] = W_eff[do*128+p, o_t*128+q]
        Wh = np.ascontiguousarray(
            W_eff.reshape(ND, P, NO, P).transpose(2, 1, 0, 3)
        ).astype(ml_dtypes.bfloat16)
        w_maps.append(Wh)

    in_maps = []
    for i in range(BS):
        # xh[do, p, s] = x[i][s, do*128+p]
        xh = np.ascontiguousarray(x[i].T.reshape(ND, P, SQ)).astype(
            ml_dtypes.bfloat16
        )
        in_maps.append({"x": xh, "w": w_maps[i // repeat], "b": bh})
    return in_maps


def kernel(x, W, b, A, B):
    if "nc" not in _CACHED:
        _CACHED["nc"] = _build()
    nc = _CACHED["nc"]

    in_maps = _prep_inputs(x, W, b, A, B)
    res = run_bass_kernel_spmd(nc, in_maps, list(range(N_CORES)))

    out = np.empty((BS, SQ, D), dtype=np.float32)
    for i in range(BS):
        yT = res.results[i]["y"].reshape(D, SQ)  # outT [o, s]
        out[i] = yT.T
    return out



# revision 2
# speedup vs baseline: 1.7731x; 1.7731x over previous
"""TRN2 Bass kernel for nn_LoRA_80839874445852.

Computes out = x @ W^T + b + (x @ A_b) @ B_b / 16 for bs=8, sq=2048, d=4096,
r=16 (per-batch LoRA skill blocks), distributed data-parallel over the batch
dim across 8 NeuronCores.

Strategy:
  * (x @ A) @ B == x @ (A @ B): the rank-16 LoRA term is folded into the
    weight on the host (cheap: 4 distinct [4096,16]@[16,4096] products), so
    each core runs a single dense GEMM:  outT = W_eff^T-contraction with xT.
  * Per core i:  W_eff_i = W.T + A_flat[i//2] @ B_flat[i//2] / 16  [d, o]
  * Mixed-precision contraction split: the first NB*128 rows of the
    contraction run as bf16 128x128 matmuls; the last NF*256 rows run as
    fp8(e4m3) DoubleRow matmuls (2 k-subtiles per pass, ~1.7x the bf16
    MACs/cycle). fp8 operands are pre-scaled on host (x*16, W*64) for
    mantissa range; the bf16 weights are pre-scaled by 1024 so both
    sections accumulate at the same scale in PSUM, and the eviction op
    applies (psum * 1/1024) + bias in a single fused tensor_scalar.
  * x is resident in SBUF (bf16 + fp8 sections); W_eff streamed per
    o-block; per-partition bias add on evict; fp32 accumulate in PSUM.
  * Host transposes outT back and stacks the full [8, 2048, 4096] output.

Accuracy: rel l2 vs fp32 reference ~1.6e-2 at NF=4, ~1.8e-2 at NF=5
(dominated by the e4m3 quantization of the fp8 section; gate is 2e-2).
"""
import numpy as np
import ml_dtypes

import concourse.bacc as bacc
import concourse.tile as tile
import concourse.mybir as mybir
from concourse.bass_utils import run_bass_kernel_spmd

# Problem shape (hardcoded per spec)
BS, SQ, D = 8, 2048, 4096
R = 16
N_CORES = 8
P = 128
ND = D // P      # 32 contraction subtiles of 128
NO = D // P      # 32 o-blocks (output features)
NS = 512         # moving free dim per matmul (PSUM bank = 512 fp32)
NST = SQ // NS   # 4 s-tiles

NF = 5           # fp8 DoubleRow pairs (each covers 2 k-subtiles = 256 rows)
NB = ND - 2 * NF # bf16 k-subtiles
KB = NB * P      # contraction rows done in bf16

SX = 16.0        # fp8 scale on x
SWF = 64.0       # fp8 scale on W
SCALE = SX * SWF # bf16 W pre-scale; evict multiplies by 1/SCALE

BF16 = mybir.dt.bfloat16
F8 = mybir.dt.float8e4
F32 = mybir.dt.float32
E4NP = ml_dtypes.float8_e4m3

_CACHED = {}


def _build(reps=1):
    """Build + compile the per-core Bass program (same program, all cores).

    reps>1 repeats the o-loop for repetition-slope timing (test harness).
    """
    nc = bacc.Bacc("TRN2", target_bir_lowering=False, debug=False)
    xb_d = nc.dram_tensor("xb", [NB, P, SQ], BF16, kind="ExternalInput").ap()
    xf_d = nc.dram_tensor("xf", [P, 2 * NF, SQ], F8, kind="ExternalInput").ap()
    wb_d = nc.dram_tensor("wb", [NO, P, NB, P], BF16, kind="ExternalInput").ap()
    wf_d = nc.dram_tensor("wf", [NO, P, 2 * NF, P], F8, kind="ExternalInput").ap()
    b_d = nc.dram_tensor("b", [P, NO], F32, kind="ExternalInput").ap()
    y_d = nc.dram_tensor("y", [NO, P, SQ], F32, kind="ExternalOutput").ap()

    with tile.TileContext(nc) as tc:
        with (
            tc.tile_pool(name="xpool", bufs=1) as xpool,
            tc.tile_pool(name="wpool", bufs=3) as wpool,
            tc.tile_pool(name="opool", bufs=3) as opool,
            tc.tile_pool(name="cpool", bufs=1) as cpool,
            tc.tile_pool(name="psum", bufs=2, space="PSUM") as psum_pool,
        ):
            bias_t = cpool.tile([P, NO], F32)
            nc.sync.dma_start(out=bias_t[:], in_=b_d[:])

            # resident x: bf16 section, NB tiles [128, 2048] (512KB each)
            xts = []
            for dtile in range(NB):
                xt = xpool.tile([P, SQ], BF16, tag=f"x{dtile}")
                nc.sync.dma_start(out=xt[:], in_=xb_d[dtile])
                xts.append(xt)
            # fp8 section: one tile [128, 2*NF, 2048]
            xft = xpool.tile([P, 2 * NF, SQ], F8, tag="xf")
            nc.sync.dma_start(out=xft[:], in_=xf_d[:])

            for rep in range(reps):
                for o in range(NO):
                    wbt = wpool.tile([P, NB, P], BF16, tag="wb")
                    nc.sync.dma_start(out=wbt[:], in_=wb_d[o])
                    wft = wpool.tile([P, 2 * NF, P], F8, tag="wf")
                    nc.sync.dma_start(out=wft[:], in_=wf_d[o])
                    ps = psum_pool.tile([P, NST, NS], F32, tag="ps")
                    for d in range(NB):
                        for s_t in range(NST):
                            nc.tensor.matmul(
                                ps[:, s_t],
                                lhsT=wbt[:, d],
                                rhs=xts[d][:, s_t * NS:(s_t + 1) * NS],
                                start=(d == 0),
                                stop=False,
                            )
                    for t in range(NF):
                        for s_t in range(NST):
                            nc.tensor.matmul(
                                ps[:, s_t],
                                lhsT=wft[:, 2 * t:2 * t + 2, :],
                                rhs=xft[:, 2 * t:2 * t + 2,
                                        s_t * NS:(s_t + 1) * NS],
                                start=False,
                                stop=(t == NF - 1),
                                perf_mode=mybir.MatmulPerfMode.DoubleRow,
                            )
                    ot = opool.tile([P, NST, NS], F32, tag="o")
                    nc.vector.tensor_scalar(
                        out=ot[:],
                        in0=ps[:],
                        scalar1=1.0 / SCALE,
                        scalar2=bias_t[:, o:o + 1],
                        op0=mybir.AluOpType.mult,
                        op1=mybir.AluOpType.add,
                    )
                    nc.sync.dma_start(
                        out=y_d[o], in_=ot[:].rearrange("p nst ns -> p (nst ns)")
                    )
    nc.compile()
    return nc


def _prep_inputs(x, W, b, A, B):
    """Host-side shard + layout prep. Returns per-core input maps."""
    x = np.asarray(x, dtype=np.float32)
    W = np.asarray(W, dtype=np.float32)
    b = np.asarray(b, dtype=np.float32)
    A = np.asarray(A, dtype=np.float32)
    B = np.asarray(B, dtype=np.float32)

    n_splits = A.shape[0]
    repeat = BS // n_splits  # 2

    # bias: [128, 32] with bh[p, o] = b[o*128 + p]
    bh = np.ascontiguousarray(b.reshape(NO, P).T)

    # distinct folded weights per skill group, in SBUF tile layout
    w_maps = []
    for g in range(n_splits):
        A_flat = A[g].reshape(D, R)                     # [d, r]
        B_flat = B[g].transpose(1, 0, 2).reshape(R, D)  # [r, o]
        W_eff = W.T + (A_flat @ B_flat) * (1.0 / R)     # [d, o]
        # bf16 section, pre-scaled by SCALE (exact power-of-2 in bf16):
        # Wb[o_t, p, u, q] = (W_eff*SCALE)[u*128+p, o_t*128+q]
        Wb = np.ascontiguousarray(
            (W_eff[:KB] * SCALE).reshape(NB, P, NO, P).transpose(2, 1, 0, 3)
        ).astype(ml_dtypes.bfloat16)
        # fp8 section: Wf[o_t, p, u, q] = q8(W_eff[KB+u*128+p, o_t*128+q]*SWF)
        Wf = np.ascontiguousarray(
            (W_eff[KB:] * SWF).reshape(2 * NF, P, NO, P).transpose(2, 1, 0, 3)
        ).astype(E4NP)
        w_maps.append((Wb, Wf))

    in_maps = []
    for i in range(BS):
        xT = x[i].T  # [d, s]
        # bf16 section: xh[u, p, s] = xT[u*128+p, s]
        xbh = np.ascontiguousarray(xT[:KB].reshape(NB, P, SQ)).astype(
            ml_dtypes.bfloat16
        )
        # fp8 section: xfh[p, u, s] = q8(xT[KB+u*128+p, s]*SX)
        xfh = np.ascontiguousarray(
            (xT[KB:] * SX).reshape(2 * NF, P, SQ).transpose(1, 0, 2)
        ).astype(E4NP)
        Wb, Wf = w_maps[i // repeat]
        in_maps.append({"xb": xbh, "xf": xfh, "wb": Wb, "wf": Wf, "b": bh})
    return in_maps


def kernel(x, W, b, A, B):
    if "nc" not in _CACHED:
        _CACHED["nc"] = _build()
    nc = _CACHED["nc"]

    in_maps = _prep_inputs(x, W, b, A, B)
    res = run_bass_kernel_spmd(nc, in_maps, list(range(N_CORES)))

    out = np.empty((BS, SQ, D), dtype=np.float32)
    for i in range(BS):
        yT = res.results[i]["y"].reshape(D, SQ)  # outT [o, s]
        out[i] = yT.T
    return out
